# revision 71
# baseline (speedup 1.0000x reference)
"""MultiHeadAttention TRN2 Bass kernel (nn_MultiHeadAttention, B=4 S=2048 E=1024 H=16).

Sharding over 8 NeuronCores: core c -> (batch b = c//2, head-half hh = c%2).
Each core computes, for its batch and its 8 heads: the Q/K/V projections,
attention, and a partial out-projection over its 512 "dk" dims with an
effective bias added; the host sums the two partials per batch (Megatron
tensor-parallel with the all-reduce replaced by a host-side pair sum).

Bias algebra (saves PE work):
  - bk is dropped entirely: k += bk shifts every score row by a constant
    (q_i . bk independent of key j) which cancels in softmax.
  - bv is folded host-side: (P @ (V + 1 bv^T))/den = PV/den + bv, so
    bo_eff = bo/2 + Wo[:, hs] @ bv[hs] and no bias is applied in V-proj.
  - bo_eff is added by the DVE during the out-proj PSUM->SBUF move from a
    partition-broadcast SBUF tile (no K=1 ones matmuls on the PE).

LDWEIGHTS reduction (PE was ~35% weight-load overhead):
  - K-proj streams each weight tile over two 512-token chunks; the second
    (identical) legalized LDWEIGHTS is removed by _dedup_ldweights.
  - Scores: one explicit full-array 128-row ldweights covers BOTH 64-row
    band loads (kt head A on PE rows 0-63, head B on 64-127);
    _dedup_ldweights removes the two covered band loads.
  - Out-proj runs per m-tile with both 512-col chunks per aout stationary.

The attention pipeline itself is deliberately the PE-heavy groups-of-3
structure: per 512-col chunk the PE (scores+PV, 2x512cy @2.4GHz) and ACT
(exp, 512cy @1.2GHz) are exactly balanced, and the PE drops to a lower
p-state (1.2GHz) if it ever micro-idles, so the schedule must keep the PE
locally over-subscribed (weight loads + interleaved Q/out-proj work).

Tail: the last attentions use a PE-side fp16 K=1 broadcast for the softmax
denominator (instead of the DRAM round-trip) and a couple of out-proj items
are held back to keep the PE busy through the final divide latency.

Measured: ~435us/core across 8 cores (baseline 460us), rel err ~5.4e-4.
"""

import numpy as np

import concourse.bass as bass
import concourse.mybir as mybir
import concourse.tile as tile
from concourse import bacc

F32 = mybir.dt.float32
F16 = mybir.dt.float16
AF = mybir.ActivationFunctionType

B, S, E, H, D = 4, 2048, 1024, 16, 64
HS = 512            # dims per core (8 heads)
PAIRS = 4           # head pairs per core
MC = 512            # m1 chunk
NMC = S // MC       # 4
NKT = E // 128      # 8 contraction chunks for projections
NMT = S // 128      # 16 m2 tiles
GROUPS = [3, 3, 3, 3, 3, 1]   # m2-tile grouping for ACT exp ops


def _ap_key(w):
    return (w.memref, w.offset, tuple(tuple(p) for p in w.ap), w.dtype)


def _dedup_ldweights(nc):
    """Remove redundant InstLdweights.

    Engines execute their instructions in block order, so after a load the
    PE array keeps those weights until the next load.  A load L is
    redundant when the previous surviving load F (with only Matmults, which
    don't disturb the array, in between on the PE) satisfies either:
      - identical AP (same memref/offset/pattern), or
      - F is a full 128-row load and L is a 64-row band of it at the
        matching tile_position row (0 -> same offset, 64 -> offset plus 64
        partition strides).
    Additionally, a 64-row band load at tile_position (0,0) directly
    followed (on the PE) by its sibling band load at (64,0) of the same
    tensor is merged: the first is widened in place to a 128-row full-array
    load and the second is dropped.

    Sync info of a removed load moves to the next kept instruction.
    """
    removed = 0
    passthrough = ("TensorCopy", "TensorScalarPtr", "TensorTensor",
                   "Activation", "DMACopy", "Memset", "ISA",
                   "EventSemaphore", "TensorReduce", "Iota", "TensorScalar")
    for fn in nc.m.functions:
        for blk in fn.blocks:
            insts = list(blk.instructions)
            keep = []
            last = None          # (memref, offset, ap, dtype) of live load
            last_inst = None     # the kept Ldweights object for widening
            pending_sync = None
            for i in insts:
                drop = False
                if i.opcode == "Ldweights":
                    w = i.ins[0]
                    key = _ap_key(w)
                    if last is not None and key == last:
                        drop = True
                    elif last is not None and last[0] == key[0]:
                        mref, off, ap, dt_ = last
                        stride = ap[0][0] if ap else None
                        tp = getattr(i, "tile_position", None)
                        ltp = getattr(last_inst, "tile_position", None) \
                            if last_inst is not None else None
                        if (len(ap) == 2 and len(key[2]) == 2
                                and dt_ == key[3]
                                and ap[1] == key[2][1]
                                and ap[0][0] == key[2][0][0]):
                            if (ap[0][1] == 128 and key[2][0][1] == 64
                                    and tp is not None
                                    and ((tp[0] == 0 and key[1] == off)
                                         or (tp[0] == 64 and key[1] == off
                                             + 64 * stride))):
                                # covered by an existing full load
                                drop = True
                            elif (ap[0][1] == 64 and key[2][0][1] == 64
                                    and ltp is not None and ltp[0] == 0
                                    and tp is not None and tp[0] == 64
                                    and key[1] == off + 64 * stride):
                                # widen the previous top-band load to a
                                # full-array load; drop this bottom one
                                lw = last_inst.ins[0]
                                lw.ap = [[stride, 128], list(ap[1])]
                                ts = getattr(last_inst, "tile_size", None)
                                if ts is not None:
                                    last_inst.tile_size = (128, ts[1])
                                last = (mref, off,
                                        ((stride, 128), tuple(ap[1])),
                                        dt_)
                                drop = True
                    if not drop:
                        last = key
                        last_inst = i
                elif i.opcode == "Matmult":
                    pass  # uses loaded weights, doesn't clobber them
                elif i.opcode in passthrough:
                    pass  # other engines don't touch the PE array
                else:
                    last = None  # control flow / drains: be conservative
                    last_inst = None
                if drop:
                    si = i.sync_info
                    if si is not None and (si.on_wait or si.on_update):
                        if pending_sync is None:
                            pending_sync = si
                        else:
                            for w_ in si.on_wait:
                                pending_sync.on_wait.append(w_)
                            for u_ in si.on_update:
                                pending_sync.on_update.append(u_)
                    removed += 1
                    continue
                if pending_sync is not None:
                    si = i.sync_info
                    if si is None:
                        i.sync_info = pending_sync
                    else:
                        for w_ in pending_sync.on_wait:
                            si.on_wait.append(w_)
                        for u_ in pending_sync.on_update:
                            si.on_update.append(u_)
                    pending_sync = None
                keep.append(i)
            if len(keep) != len(insts):
                blk.instructions = keep
    return removed


def build_nc():
    nc = bacc.Bacc()

    xq_d = nc.dram_tensor("xq_t", [E, S], F16, kind="ExternalInput")
    xk_d = nc.dram_tensor("xk_t", [E, S], F16, kind="ExternalInput")
    xv_d = nc.dram_tensor("xv_t", [E, S], F16, kind="ExternalInput")
    wq_d = nc.dram_tensor("wq_t", [E, HS], F16, kind="ExternalInput")
    wk_d = nc.dram_tensor("wk_t", [E, HS], F16, kind="ExternalInput")
    wv_d = nc.dram_tensor("wv_t", [E, HS], F16, kind="ExternalInput")
    wo_d = nc.dram_tensor("wo_t", [HS, E], F16, kind="ExternalInput")
    bq_d = nc.dram_tensor("bq", [HS], F32, kind="ExternalInput")
    bo_d = nc.dram_tensor("bo_row", [1, E], F32, kind="ExternalInput")

    out_d = nc.dram_tensor("out_partial", [S, E], F32, kind="ExternalOutput")
    scratch_d = nc.dram_tensor("scratch_w1", [NMC, PAIRS, 2, MC], F32)

    def bcast_ap(row_ap, n):
        return bass.AP(tensor=row_ap.tensor, offset=row_ap.offset,
                       ap=[[0, n]] + list(row_ap.ap[1:]))

    with tile.TileContext(nc) as tc:
        with (
            tc.tile_pool(name="const", bufs=1) as const,
            tc.tile_pool(name="qkv", bufs=1) as qkv,
            tc.tile_pool(name="aout", bufs=1) as aoutp,
        ):
            bq_sb = const.tile([128, PAIRS], F32)
            bo_bc = const.tile([128, E], F32)
            ones64 = const.tile([128, 64], F16)
            nc.vector.memset(ones64[:], 1.0)

            qt_all = qkv.tile([128, PAIRS, S], F16, tag="qt")
            kt_all = qkv.tile([128, PAIRS, S], F16, tag="kt")
            v_all = qkv.tile([128, NMT, 8, 65], F16, tag="v")
            nc.vector.memset(v_all[:, :, :, 64], 1.0)

            aout = [aoutp.tile([128, S], F16, name=f"aout{p}", tag=f"ao{p}")
                    for p in range(PAIRS)]

            with (
                tc.tile_pool(name="w", bufs=2) as wpool,
                tc.tile_pool(name="x", bufs=2) as xpool,
            ):
                # ======== K and V projections (own PSUM scope) ========
                with tc.tile_pool(name="pp", bufs=2,
                                  space=bass.MemorySpace.PSUM) as pp:
                    # PE warm-up: dependency-free matmuls on memset data run
                    # while the first DMAs are in flight, ramping the PE out
                    # of its cold p-state (0.65GHz) so the first real
                    # matmuls stream at 2.4GHz.
                    warm_ps = pp.tile([64, HS], F32, tag="ppv", name="warm",
                                      bufs=4)
                    for i in range(20):
                        # alternate K 128/127 so the identical loads aren't
                        # collapsed by _dedup_ldweights
                        k = 128 - (i % 2)
                        nc.tensor.matmul(
                            warm_ps[:, 0:64], ones64[0:k, :], ones64[0:k, :],
                            start=True, stop=True,
                        )

                    wk_sb = wpool.tile([128, NKT, HS], F16, tag="w")
                    wk_r = wk_d.rearrange("(kc p) n -> p kc n", p=128)
                    xk_r = xk_d.rearrange("(kc p) m -> p kc m", p=128)

                    for mch in range(NMC // 2):
                        msl = slice(mch * 2 * MC, (mch + 1) * 2 * MC)
                        x_t = xpool.tile([128, NKT, 2 * MC], F16, tag="x")
                        if mch == 0:
                            # first-use DMAs split per-kc in consumption
                            # order: the first matmul can start after ~1/8
                            # of the data lands
                            for kc in range(NKT):
                                nc.sync.dma_start(wk_sb[:, kc, :],
                                                  wk_r[:, kc, :])
                                nc.sync.dma_start(x_t[:, kc, :],
                                                  xk_r[:, kc, msl])
                        else:
                            nc.sync.dma_start(x_t[:], xk_r[:, :, msl])
                        if mch == 0:
                            # deferred small/constant loads: keep the first
                            # compute DMAs at the head of the queues
                            nc.sync.dma_start(
                                bq_sb[:],
                                bq_d.rearrange("(t p) -> p t", p=128))
                            nc.sync.dma_start(bo_bc[:],
                                              bcast_ap(bo_d[:], 128))
                        for nt in range(PAIRS):
                            ps = pp.tile([128, 2, MC], F32, tag="ppk")
                            for kc in range(NKT):
                                for h in range(2):
                                    nc.tensor.matmul(
                                        ps[:, h, :],
                                        wk_sb[:, kc, nt * 128:(nt + 1) * 128],
                                        x_t[:, kc, h * MC:(h + 1) * MC],
                                        start=(kc == 0),
                                        stop=(kc == NKT - 1),
                                    )
                            nc.vector.tensor_copy(
                                kt_all[:, nt, msl], ps[:])

                    wv_sb = wpool.tile([128, NKT, HS], F16, tag="w")
                    nc.sync.dma_start(
                        wv_sb[:], wv_d.rearrange("(kc p) n -> p kc n", p=128))
                    for mch in range(NMC // 2):
                        msl = slice(mch * 2 * MC, (mch + 1) * 2 * MC)
                        x_t = xpool.tile([128, NKT, 2 * MC], F16, tag="x")
                        nc.sync.dma_start(
                            x_t[:],
                            xv_d.rearrange("(kc p) m -> p kc m", p=128)[
                                :, :, msl
                            ],
                        )
                        for mt_l in range(2 * MC // 128):
                            mt = mch * (2 * MC // 128) + mt_l
                            ps = pp.tile([128, HS], F32, tag="ppv", bufs=4)
                            for kc in range(NKT):
                                nc.tensor.matmul(
                                    ps[:],
                                    x_t[:, kc, mt_l * 128:(mt_l + 1) * 128],
                                    wv_sb[:, kc, :],
                                    start=(kc == 0),
                                    stop=(kc == NKT - 1),
                                )
                            nc.vector.tensor_copy(
                                v_all[:, mt, :, 0:64],
                                ps[:].rearrange("p (h c) -> p h c", c=64),
                            )

                # ======== Q-proj interleaved with attention + out-proj ====
                wq_sb = wpool.tile([128, NKT, HS], F16, tag="w")
                nc.sync.dma_start(
                    wq_sb[:], wq_d.rearrange("(kc p) n -> p kc n", p=128))

                with (
                    tc.tile_pool(name="wo", bufs=1) as wop,
                    tc.tile_pool(name="pt", bufs=4) as ptp,
                    tc.tile_pool(name="msc", bufs=2) as msc,
                    tc.tile_pool(name="ost", bufs=4) as ostp,
                    tc.tile_pool(name="sc", bufs=2,
                                 space=bass.MemorySpace.PSUM) as scp,
                    tc.tile_pool(name="pv", bufs=2,
                                 space=bass.MemorySpace.PSUM) as pvp,
                ):
                    wo_sb = wop.tile([128, PAIRS, E], F16, tag="wo")
                    nc.sync.dma_start(
                        wo_sb[:], wo_d.rearrange("(dk p) n -> p dk n", p=128))

                    qx = {}

                    def qproj_x(mh):
                        x_t = xpool.tile([128, NKT, 2 * MC], F16, tag="x",
                                         name=f"xq{mh}")
                        nc.sync.dma_start(
                            x_t[:],
                            xq_d.rearrange("(kc p) m -> p kc m", p=128)[
                                :, :, mh * 2 * MC:(mh + 1) * 2 * MC
                            ],
                        )
                        qx[mh] = x_t

                    def qproj_group(mh, nt):
                        # compound over a 1024-token chunk: both halves
                        # write one tile, keeping the same-weights matmuls
                        # adjacent so the second LDWEIGHTS dedupes
                        x_t = qx[mh]
                        ps = scp.tile([128, 2, MC], F32, tag="sc",
                                      name="qps")
                        for kc in range(NKT):
                            for h in range(2):
                                nc.tensor.matmul(
                                    ps[:, h, :],
                                    wq_sb[:, kc, nt * 128:(nt + 1) * 128],
                                    x_t[:, kc, h * MC:(h + 1) * MC],
                                    start=(kc == 0),
                                    stop=(kc == NKT - 1),
                                )
                        nc.vector.tensor_scalar_add(
                            qt_all[:, nt, mh * 2 * MC:(mh + 1) * 2 * MC],
                            ps[:],
                            bq_sb[:, nt:nt + 1],
                        )

                    def attention(mc, pair, extra=(), fast_divide=False):
                        extra = list(extra)
                        m1 = slice(mc * MC, (mc + 1) * MC)
                        pvt = [pvp.tile([128, MC], F32, name=f"pv{h}",
                                        tag="pv") for h in range(2)]
                        mt0 = 0
                        for gidx, gsize in enumerate(GROUPS):
                            scA = scp.tile([128, 3, MC], F32, tag="sc")
                            scB = scp.tile([128, 3, MC], F32, tag="sc")
                            for gi in range(gsize):
                                t = mt0 + gi
                                m2 = slice(t * 128, (t + 1) * 128)
                                # the two band loads (rows 0-63 / 64-127)
                                # are merged into one 128-row load by
                                # _dedup_ldweights
                                nc.tensor.matmul(
                                    scA[:, gi, :],
                                    kt_all[0:64, pair, m2],
                                    qt_all[0:64, pair, m1],
                                    start=True, stop=True,
                                    tile_position=(0, 0),
                                )
                                nc.tensor.matmul(
                                    scB[:, gi, :],
                                    kt_all[64:128, pair, m2],
                                    qt_all[64:128, pair, m1],
                                    start=True, stop=True,
                                    tile_position=(64, 0),
                                )
                            ptA = ptp.tile([128, 3, MC], F16, tag="pt")
                            ptB = ptp.tile([128, 3, MC], F16, tag="pt")
                            nc.scalar.activation(
                                ptA[:, 0:gsize, :], scA[:, 0:gsize, :],
                                AF.Exp, scale=0.125,
                            )
                            nc.scalar.activation(
                                ptB[:, 0:gsize, :], scB[:, 0:gsize, :],
                                AF.Exp, scale=0.125,
                            )
                            for gi in range(gsize):
                                t = mt0 + gi
                                nc.tensor.matmul(
                                    pvt[0][0:65, :],
                                    v_all[:, t, 2 * pair, :],
                                    ptA[:, gi, :],
                                    start=(t == 0), stop=(t == NMT - 1),
                                )
                                nc.tensor.matmul(
                                    pvt[1][0:65, :],
                                    v_all[:, t, 2 * pair + 1, :],
                                    ptB[:, gi, :],
                                    start=(t == 0), stop=(t == NMT - 1),
                                )
                            mt0 += gsize
                            # interleave one spread-work item (out-proj or
                            # Q-proj group) between m2-groups so PE and ACT
                            # stay fed through pair and chunk boundaries
                            if gidx % 2 == 1 and extra:
                                extra.pop(0)()
                        while extra:
                            extra.pop(0)()

                        # normalize: out_h = pv[0:64] / pv[64].  Copy
                        # PSUM->SBUF immediately (frees the pv bank for the
                        # next pair), then divide from the SBUF copy.
                        # h==1 first: its aout write ends in a partition-
                        # shift DMA, so start that chain earlier.
                        for h in (1, 0):
                            pvs = msc.tile([128, MC], F32, name=f"pvs{h}",
                                           tag="pvs")
                            nc.vector.tensor_copy(pvs[0:65, :],
                                                  pvt[h][0:65, :])
                            inv = msc.tile([64, MC], F32, tag="inv")
                            if fast_divide:
                                # tail-latency path: broadcast the den row
                                # with a K=1 fp16 PE outer product instead
                                # of the DRAM round-trip (fp16: an fp32
                                # matmul here downclocked the whole chip)
                                den16 = msc.tile([128, MC], F16,
                                                 name="den16", tag="den16")
                                nc.vector.tensor_copy(den16[64:65, :],
                                                      pvs[64:65, :])
                                bc_ps = pvp.tile([64, MC], F32, tag="pv",
                                                 name="bcp")
                                nc.tensor.matmul(
                                    bc_ps[:], ones64[64:65, :],
                                    den16[64:65, :],
                                    start=True, stop=True,
                                    tile_position=(64, 0),
                                )
                                nc.vector.reciprocal_approx_fast(inv[:],
                                                                 bc_ps[:])
                            else:
                                srow_dram = scratch_d[mc:mc + 1, pair, h, :]
                                nc.sync.dma_start(srow_dram, pvs[64:65, :])
                                bc = msc.tile([64, MC], F32, tag="bc")
                                nc.sync.dma_start(bc[:],
                                                  bcast_ap(srow_dram, 64))
                                nc.vector.reciprocal_approx_fast(inv[:],
                                                                 bc[:])
                            if h == 0:
                                nc.vector.tensor_mul(
                                    aout[pair][0:64, m1], pvs[0:64, :],
                                    inv[:],
                                )
                            else:
                                tmpb = msc.tile([64, MC], F16, tag="tmpb")
                                nc.vector.tensor_mul(tmpb[:], pvs[0:64, :],
                                                     inv[:])
                                nc.sync.dma_start(aout[pair][64:128, m1],
                                                  tmpb[:])

                    def outproj_item(mt, nch):
                        msl = slice(mt * 128, (mt + 1) * 128)
                        nsl = slice(nch * 512, (nch + 1) * 512)
                        ps = pvp.tile([128, 512], F32, tag="pv", name="op")
                        for dk in range(PAIRS):
                            nc.tensor.matmul(
                                ps[:],
                                aout[dk][:, msl],
                                wo_sb[:, dk, nsl],
                                start=(dk == 0),
                                stop=(dk == PAIRS - 1),
                            )
                        ost = ostp.tile([128, 512], F32, tag="ost")
                        nc.vector.tensor_add(ost[:], ps[:], bo_bc[:, nsl])
                        nc.sync.dma_start(out_d[msl, nsl], ost[:])

                    og = [(m, n) for m in range(MC // 128)
                          for n in range(2)]
                    qproj_x(0)
                    for nt in range(PAIRS):
                        qproj_group(0, nt)
                    leftover = []
                    for mc in range(NMC):
                        for pair in range(PAIRS):
                            if pair == 0 and mc == 0:
                                qproj_x(1)
                            work = []
                            if mc >= 1:
                                items = og[2 * pair:2 * pair + 2]
                                for k, (m_l, n_) in enumerate(items):
                                    mt = (mc - 1) * (MC // 128) + m_l
                                    fn = (lambda mtt=mt, nn=n_:
                                          outproj_item(mtt, nn))
                                    # reserve a couple of items to keep the
                                    # PE fed through the final divide
                                    if mc == NMC - 1 and pair >= 2 and k:
                                        leftover.append(fn)
                                    else:
                                        work.append(fn)
                            if mc == 0:
                                work.append(
                                    lambda nt=pair: qproj_group(1, nt))
                            attention(mc, pair, work,
                                      fast_divide=(mc == NMC - 1
                                                   and pair >= PAIRS - 2))
                    for fn in leftover:
                        fn()
                    for m_l in range(MC // 128):
                        for n_ in range(2):
                            outproj_item(12 + m_l, n_)

    n = _dedup_ldweights(nc)
    print(f"dedup_ldweights removed {n}")
    return nc


def kernel(**inputs):
    query = np.asarray(inputs["query"], np.float32)
    key = np.asarray(inputs["key"], np.float32)
    value = np.asarray(inputs["value"], np.float32)
    Wq = np.asarray(inputs["Wq"], np.float32)
    bq = np.asarray(inputs["bq"], np.float32)
    Wk = np.asarray(inputs["Wk"], np.float32)
    Wv = np.asarray(inputs["Wv"], np.float32)
    bv = np.asarray(inputs["bv"], np.float32)
    Wo = np.asarray(inputs["Wo"], np.float32)
    bo = np.asarray(inputs["bo"], np.float32)

    nc = build_nc()

    in_maps = []
    for c in range(8):
        b, hh = c // 2, c % 2
        hs = slice(hh * HS, (hh + 1) * HS)

        def prep(a):
            return np.ascontiguousarray(a).astype(np.float16)

        bo_eff = bo * 0.5 + Wo[:, hs] @ bv[hs]
        in_maps.append({
            "xq_t": prep(query[b].T),
            "xk_t": prep(key[b].T),
            "xv_t": prep(value[b].T),
            "wq_t": prep(Wq[hs, :].T),
            "wk_t": prep(Wk[hs, :].T),
            "wv_t": prep(Wv[hs, :].T),
            "wo_t": prep(Wo[:, hs].T),
            "bq": np.ascontiguousarray(bq[hs]),
            "bo_row": bo_eff.reshape(1, E).astype(np.float32),
        })

    from concourse.bass_utils import run_bass_kernel_spmd
    nc.finalize()
    r = run_bass_kernel_spmd(nc, in_maps, core_ids=list(range(8)))
    globals()["LAST_RUN"] = r
    outs = [r.results[c]["out_partial"] for c in range(8)]
    return np.stack([outs[2 * b] + outs[2 * b + 1] for b in range(B)])


# revision 72
# speedup vs baseline: 1.1524x; 1.1524x over previous
"""MultiHeadAttention TRN2 Bass kernel (nn_MultiHeadAttention, B=4 S=2048 E=1024 H=16).

Sharding over 8 NeuronCores: core c -> (batch b = c//2, head-half hh = c%2).
Each core computes, for its batch and its 8 heads: the Q/K/V projections,
attention, and a partial out-projection over its 512 "dk" dims with an
effective bias added; the host sums the two partials per batch (Megatron
tensor-parallel with the all-reduce replaced by a host-side pair sum).

Bias algebra (saves PE work):
  - bk is dropped entirely: k += bk shifts every score row by a constant
    (q_i . bk independent of key j) which cancels in softmax.
  - bv is folded host-side: (P @ (V + 1 bv^T))/den = PV/den + bv, so
    bo_eff = bo/2 + Wo[:, hs] @ bv[hs] and no bias is applied in V-proj.
  - bo_eff is added by the DVE during the out-proj PSUM->SBUF move from a
    partition-broadcast SBUF tile (no K=1 ones matmuls on the PE).

LDWEIGHTS reduction (PE was ~35% weight-load overhead):
  - K-proj streams each weight tile over two 512-token chunks; the second
    (identical) legalized LDWEIGHTS is removed by _dedup_ldweights.
  - Scores: one explicit full-array 128-row ldweights covers BOTH 64-row
    band loads (kt head A on PE rows 0-63, head B on 64-127);
    _dedup_ldweights removes the two covered band loads.
  - Out-proj runs per m-tile with both 512-col chunks per aout stationary.

The attention pipeline itself is deliberately the PE-heavy groups-of-3
structure: per 512-col chunk the PE (scores+PV, 2x512cy @2.4GHz) and ACT
(exp, 512cy @1.2GHz) are exactly balanced, and the PE drops to a lower
p-state (1.2GHz) if it ever micro-idles, so the schedule must keep the PE
locally over-subscribed (weight loads + interleaved Q/out-proj work).

Tail: the last attentions use a PE-side fp16 K=1 broadcast for the softmax
denominator (instead of the DRAM round-trip) and a couple of out-proj items
are held back to keep the PE busy through the final divide latency.

Measured: ~435us/core across 8 cores (baseline 460us), rel err ~5.4e-4.
"""

import numpy as np

import concourse.bass as bass
import concourse.mybir as mybir
import concourse.tile as tile
from concourse import bacc

F32 = mybir.dt.float32
F16 = mybir.dt.float16
AF = mybir.ActivationFunctionType

B, S, E, H, D = 4, 2048, 1024, 16, 64
HS = 512            # dims per core (8 heads)
PAIRS = 4           # head pairs per core
MC = 512            # m1 chunk
NMC = S // MC       # 4
NKT = E // 128      # 8 contraction chunks for projections
NMT = S // 128      # 16 m2 tiles
GROUPS = [3, 3, 3, 3, 3, 1]   # m2-tile grouping for ACT exp ops


def _ap_key(w):
    return (w.memref, w.offset, tuple(tuple(p) for p in w.ap), w.dtype)


def _dedup_ldweights(nc):
    """Remove redundant InstLdweights.

    Engines execute their instructions in block order, so after a load the
    PE array keeps those weights until the next load.  A load L is
    redundant when the previous surviving load F (with only Matmults, which
    don't disturb the array, in between on the PE) satisfies either:
      - identical AP (same memref/offset/pattern), or
      - F is a full 128-row load and L is a 64-row band of it at the
        matching tile_position row (0 -> same offset, 64 -> offset plus 64
        partition strides).
    Additionally, a 64-row band load at tile_position (0,0) directly
    followed (on the PE) by its sibling band load at (64,0) of the same
    tensor is merged: the first is widened in place to a 128-row full-array
    load and the second is dropped.

    Sync info of a removed load moves to the next kept instruction.
    """
    removed = 0
    passthrough = ("TensorCopy", "TensorScalarPtr", "TensorTensor",
                   "Activation", "DMACopy", "Memset", "ISA",
                   "EventSemaphore", "TensorReduce", "Iota", "TensorScalar")
    for fn in nc.m.functions:
        for blk in fn.blocks:
            insts = list(blk.instructions)
            keep = []
            last = None          # (memref, offset, ap, dtype) of live load
            last_inst = None     # the kept Ldweights object for widening
            pending_sync = None
            for i in insts:
                drop = False
                if i.opcode == "Ldweights":
                    w = i.ins[0]
                    key = _ap_key(w)
                    if last is not None and key == last:
                        drop = True
                    elif last is not None and last[0] == key[0]:
                        mref, off, ap, dt_ = last
                        stride = ap[0][0] if ap else None
                        tp = getattr(i, "tile_position", None)
                        ltp = getattr(last_inst, "tile_position", None) \
                            if last_inst is not None else None
                        if (len(ap) == 2 and len(key[2]) == 2
                                and dt_ == key[3]
                                and ap[1] == key[2][1]
                                and ap[0][0] == key[2][0][0]):
                            if (ap[0][1] == 128 and key[2][0][1] == 64
                                    and tp is not None
                                    and ((tp[0] == 0 and key[1] == off)
                                         or (tp[0] == 64 and key[1] == off
                                             + 64 * stride))):
                                # covered by an existing full load
                                drop = True
                            elif (ap[0][1] == 64 and key[2][0][1] == 64
                                    and ltp is not None and ltp[0] == 0
                                    and tp is not None and tp[0] == 64
                                    and key[1] == off + 64 * stride):
                                # widen the previous top-band load to a
                                # full-array load; drop this bottom one
                                lw = last_inst.ins[0]
                                lw.ap = [[stride, 128], list(ap[1])]
                                ts = getattr(last_inst, "tile_size", None)
                                if ts is not None:
                                    last_inst.tile_size = (128, ts[1])
                                last = (mref, off,
                                        ((stride, 128), tuple(ap[1])),
                                        dt_)
                                drop = True
                    if not drop:
                        last = key
                        last_inst = i
                elif i.opcode == "Matmult":
                    pass  # uses loaded weights, doesn't clobber them
                elif i.opcode in passthrough:
                    pass  # other engines don't touch the PE array
                else:
                    last = None  # control flow / drains: be conservative
                    last_inst = None
                if drop:
                    si = i.sync_info
                    if si is not None and (si.on_wait or si.on_update):
                        if pending_sync is None:
                            pending_sync = si
                        else:
                            for w_ in si.on_wait:
                                pending_sync.on_wait.append(w_)
                            for u_ in si.on_update:
                                pending_sync.on_update.append(u_)
                    removed += 1
                    continue
                if pending_sync is not None:
                    si = i.sync_info
                    if si is None:
                        i.sync_info = pending_sync
                    else:
                        for w_ in pending_sync.on_wait:
                            si.on_wait.append(w_)
                        for u_ in pending_sync.on_update:
                            si.on_update.append(u_)
                    pending_sync = None
                keep.append(i)
            if len(keep) != len(insts):
                blk.instructions = keep
    return removed


def build_nc():
    nc = bacc.Bacc()

    xq_d = nc.dram_tensor("xq_t", [E, S], F16, kind="ExternalInput")
    xk_d = nc.dram_tensor("xk_t", [E, S], F16, kind="ExternalInput")
    xv_d = nc.dram_tensor("xv_t", [E, S], F16, kind="ExternalInput")
    wq_d = nc.dram_tensor("wq_t", [E, HS], F16, kind="ExternalInput")
    wk_d = nc.dram_tensor("wk_t", [E, HS], F16, kind="ExternalInput")
    wv_d = nc.dram_tensor("wv_t", [E, HS], F16, kind="ExternalInput")
    wo_d = nc.dram_tensor("wo_t", [HS, E], F16, kind="ExternalInput")
    bq_d = nc.dram_tensor("bq", [HS], F32, kind="ExternalInput")
    bo_d = nc.dram_tensor("bo_row", [1, E], F32, kind="ExternalInput")

    out_d = nc.dram_tensor("out_partial", [S, E], F32, kind="ExternalOutput")
    scratch_d = nc.dram_tensor("scratch_w1", [NMC, PAIRS, 2, MC], F32)

    def bcast_ap(row_ap, n):
        return bass.AP(tensor=row_ap.tensor, offset=row_ap.offset,
                       ap=[[0, n]] + list(row_ap.ap[1:]))

    with tile.TileContext(nc) as tc:
        with (
            tc.tile_pool(name="const", bufs=1) as const,
            tc.tile_pool(name="qkv", bufs=1) as qkv,
            tc.tile_pool(name="aout", bufs=1) as aoutp,
        ):
            bq_sb = const.tile([128, PAIRS], F32)
            bo_bc = const.tile([128, E], F32)
            ones64 = const.tile([128, 64], F16)
            nc.vector.memset(ones64[:], 1.0)

            qt_all = qkv.tile([128, PAIRS, S], F16, tag="qt")
            kt_all = qkv.tile([128, PAIRS, S], F16, tag="kt")
            v_all = qkv.tile([128, NMT, 8, 65], F16, tag="v")
            nc.vector.memset(v_all[:, :, :, 64], 1.0)

            aout = [aoutp.tile([128, S], F16, name=f"aout{p}", tag=f"ao{p}")
                    for p in range(PAIRS)]

            with (
                tc.tile_pool(name="w", bufs=2) as wpool,
                tc.tile_pool(name="x", bufs=2) as xpool,
            ):
                # ======== K and V projections (own PSUM scope) ========
                with tc.tile_pool(name="pp", bufs=2,
                                  space=bass.MemorySpace.PSUM) as pp:
                    # PE warm-up: dependency-free matmuls on memset data run
                    # while the first DMAs are in flight, ramping the PE out
                    # of its cold p-state (0.65GHz) so the first real
                    # matmuls stream at 2.4GHz.
                    warm_ps = pp.tile([64, HS], F32, tag="ppv", name="warm",
                                      bufs=4)
                    for i in range(20):
                        # alternate K 128/127 so the identical loads aren't
                        # collapsed by _dedup_ldweights
                        k = 128 - (i % 2)
                        nc.tensor.matmul(
                            warm_ps[:, 0:64], ones64[0:k, :], ones64[0:k, :],
                            start=True, stop=True,
                        )

                    wk_sb = wpool.tile([128, NKT, HS], F16, tag="w")
                    wk_r = wk_d.rearrange("(kc p) n -> p kc n", p=128)
                    xk_r = xk_d.rearrange("(kc p) m -> p kc m", p=128)

                    for mch in range(NMC // 2):
                        msl = slice(mch * 2 * MC, (mch + 1) * 2 * MC)
                        x_t = xpool.tile([128, NKT, 2 * MC], F16, tag="x")
                        if mch == 0:
                            # first-use DMAs split per-kc in consumption
                            # order: the first matmul can start after ~1/8
                            # of the data lands
                            for kc in range(NKT):
                                nc.sync.dma_start(wk_sb[:, kc, :],
                                                  wk_r[:, kc, :])
                                nc.sync.dma_start(x_t[:, kc, :],
                                                  xk_r[:, kc, msl])
                        else:
                            nc.sync.dma_start(x_t[:], xk_r[:, :, msl])
                        if mch == 0:
                            # deferred small/constant loads: keep the first
                            # compute DMAs at the head of the queues
                            nc.sync.dma_start(
                                bq_sb[:],
                                bq_d.rearrange("(t p) -> p t", p=128))
                            nc.sync.dma_start(bo_bc[:],
                                              bcast_ap(bo_d[:], 128))
                        for nt in range(PAIRS):
                            ps = pp.tile([128, 2, MC], F32, tag="ppk")
                            for kc in range(NKT):
                                for h in range(2):
                                    nc.tensor.matmul(
                                        ps[:, h, :],
                                        wk_sb[:, kc, nt * 128:(nt + 1) * 128],
                                        x_t[:, kc, h * MC:(h + 1) * MC],
                                        start=(kc == 0),
                                        stop=(kc == NKT - 1),
                                    )
                            nc.vector.tensor_copy(
                                kt_all[:, nt, msl], ps[:])

                    wv_sb = wpool.tile([128, NKT, HS], F16, tag="w")
                    nc.sync.dma_start(
                        wv_sb[:], wv_d.rearrange("(kc p) n -> p kc n", p=128))
                    for mch in range(NMC // 2):
                        msl = slice(mch * 2 * MC, (mch + 1) * 2 * MC)
                        x_t = xpool.tile([128, NKT, 2 * MC], F16, tag="x")
                        nc.sync.dma_start(
                            x_t[:],
                            xv_d.rearrange("(kc p) m -> p kc m", p=128)[
                                :, :, msl
                            ],
                        )
                        for mt_l in range(2 * MC // 128):
                            mt = mch * (2 * MC // 128) + mt_l
                            ps = pp.tile([128, HS], F32, tag="ppv", bufs=4)
                            for kc in range(NKT):
                                nc.tensor.matmul(
                                    ps[:],
                                    x_t[:, kc, mt_l * 128:(mt_l + 1) * 128],
                                    wv_sb[:, kc, :],
                                    start=(kc == 0),
                                    stop=(kc == NKT - 1),
                                )
                            nc.vector.tensor_copy(
                                v_all[:, mt, :, 0:64],
                                ps[:].rearrange("p (h c) -> p h c", c=64),
                            )

                # ======== Q-proj interleaved with attention + out-proj ====
                wq_sb = wpool.tile([128, NKT, HS], F16, tag="w")
                nc.sync.dma_start(
                    wq_sb[:], wq_d.rearrange("(kc p) n -> p kc n", p=128))

                with (
                    tc.tile_pool(name="wo", bufs=1) as wop,
                    tc.tile_pool(name="pt", bufs=4) as ptp,
                    tc.tile_pool(name="msc", bufs=2) as msc,
                    tc.tile_pool(name="ost", bufs=4) as ostp,
                    tc.tile_pool(name="sc", bufs=2,
                                 space=bass.MemorySpace.PSUM) as scp,
                    tc.tile_pool(name="pv", bufs=2,
                                 space=bass.MemorySpace.PSUM) as pvp,
                ):
                    wo_sb = wop.tile([128, PAIRS, E], F16, tag="wo")
                    nc.sync.dma_start(
                        wo_sb[:], wo_d.rearrange("(dk p) n -> p dk n", p=128))

                    qx = {}

                    def qproj_x(mh):
                        x_t = xpool.tile([128, NKT, 2 * MC], F16, tag="x",
                                         name=f"xq{mh}")
                        nc.sync.dma_start(
                            x_t[:],
                            xq_d.rearrange("(kc p) m -> p kc m", p=128)[
                                :, :, mh * 2 * MC:(mh + 1) * 2 * MC
                            ],
                        )
                        qx[mh] = x_t

                    def qproj_group(mh, nt):
                        # compound over a 1024-token chunk: both halves
                        # write one tile, keeping the same-weights matmuls
                        # adjacent so the second LDWEIGHTS dedupes
                        x_t = qx[mh]
                        ps = scp.tile([128, 2, MC], F32, tag="sc",
                                      name="qps")
                        for kc in range(NKT):
                            for h in range(2):
                                nc.tensor.matmul(
                                    ps[:, h, :],
                                    wq_sb[:, kc, nt * 128:(nt + 1) * 128],
                                    x_t[:, kc, h * MC:(h + 1) * MC],
                                    start=(kc == 0),
                                    stop=(kc == NKT - 1),
                                )
                        nc.vector.tensor_scalar_add(
                            qt_all[:, nt, mh * 2 * MC:(mh + 1) * 2 * MC],
                            ps[:],
                            bq_sb[:, nt:nt + 1],
                        )

                    def attention(mc, pair, extra=(), fast_divide=False):
                        extra = list(extra)
                        m1 = slice(mc * MC, (mc + 1) * MC)
                        pvt = [pvp.tile([128, MC], F32, name=f"pv{h}",
                                        tag="pv") for h in range(2)]
                        mt0 = 0
                        for gidx, gsize in enumerate(GROUPS):
                            scA = scp.tile([128, 3, MC], F32, tag="sc")
                            scB = scp.tile([128, 3, MC], F32, tag="sc")
                            for gi in range(gsize):
                                t = mt0 + gi
                                m2 = slice(t * 128, (t + 1) * 128)
                                # the two band loads (rows 0-63 / 64-127)
                                # are merged into one 128-row load by
                                # _dedup_ldweights
                                nc.tensor.matmul(
                                    scA[:, gi, :],
                                    kt_all[0:64, pair, m2],
                                    qt_all[0:64, pair, m1],
                                    start=True, stop=True,
                                    tile_position=(0, 0),
                                )
                                nc.tensor.matmul(
                                    scB[:, gi, :],
                                    kt_all[64:128, pair, m2],
                                    qt_all[64:128, pair, m1],
                                    start=True, stop=True,
                                    tile_position=(64, 0),
                                )
                            ptA = ptp.tile([128, 3, MC], F16, tag="pt")
                            ptB = ptp.tile([128, 3, MC], F16, tag="pt")
                            nc.scalar.activation(
                                ptA[:, 0:gsize, :], scA[:, 0:gsize, :],
                                AF.Exp, scale=0.125,
                            )
                            nc.scalar.activation(
                                ptB[:, 0:gsize, :], scB[:, 0:gsize, :],
                                AF.Exp, scale=0.125,
                            )
                            for gi in range(gsize):
                                t = mt0 + gi
                                nc.tensor.matmul(
                                    pvt[0][0:65, :],
                                    v_all[:, t, 2 * pair, :],
                                    ptA[:, gi, :],
                                    start=(t == 0), stop=(t == NMT - 1),
                                )
                                nc.tensor.matmul(
                                    pvt[1][0:65, :],
                                    v_all[:, t, 2 * pair + 1, :],
                                    ptB[:, gi, :],
                                    start=(t == 0), stop=(t == NMT - 1),
                                )
                            mt0 += gsize
                            # interleave one spread-work item (out-proj or
                            # Q-proj group) between m2-groups so PE and ACT
                            # stay fed through pair and chunk boundaries
                            if gidx % 2 == 1 and extra:
                                extra.pop(0)()
                        while extra:
                            extra.pop(0)()

                        # normalize: out_h = pv[0:64] / pv[64].  Copy
                        # PSUM->SBUF immediately (frees the pv bank for the
                        # next pair), then divide from the SBUF copy.
                        for h in range(2):
                            pvs = msc.tile([128, MC], F32, name=f"pvs{h}",
                                           tag="pvs")
                            nc.vector.tensor_copy(pvs[0:65, :],
                                                  pvt[h][0:65, :])
                            inv = msc.tile([64, MC], F32, tag="inv")
                            if fast_divide:
                                # tail-latency path: broadcast the den row
                                # with a K=1 fp16 PE outer product instead
                                # of the DRAM round-trip (fp16: an fp32
                                # matmul here downclocked the whole chip)
                                den16 = msc.tile([128, MC], F16,
                                                 name="den16", tag="den16")
                                nc.vector.tensor_copy(den16[64:65, :],
                                                      pvs[64:65, :])
                                bc_ps = pvp.tile([64, MC], F32, tag="pv",
                                                 name="bcp")
                                nc.tensor.matmul(
                                    bc_ps[:], ones64[64:65, :],
                                    den16[64:65, :],
                                    start=True, stop=True,
                                    tile_position=(64, 0),
                                )
                                nc.vector.reciprocal_approx_fast(inv[:],
                                                                 bc_ps[:])
                            else:
                                srow_dram = scratch_d[mc:mc + 1, pair, h, :]
                                nc.sync.dma_start(srow_dram, pvs[64:65, :])
                                bc = msc.tile([64, MC], F32, tag="bc")
                                nc.sync.dma_start(bc[:],
                                                  bcast_ap(srow_dram, 64))
                                nc.vector.reciprocal_approx_fast(inv[:],
                                                                 bc[:])
                            if h == 0:
                                nc.vector.tensor_mul(
                                    aout[pair][0:64, m1], pvs[0:64, :],
                                    inv[:],
                                )
                            else:
                                tmpb = msc.tile([64, MC], F16, tag="tmpb")
                                nc.vector.tensor_mul(tmpb[:], pvs[0:64, :],
                                                     inv[:])
                                nc.sync.dma_start(aout[pair][64:128, m1],
                                                  tmpb[:])

                    def outproj_item(mt, nch):
                        msl = slice(mt * 128, (mt + 1) * 128)
                        nsl = slice(nch * 512, (nch + 1) * 512)
                        ps = pvp.tile([128, 512], F32, tag="pv", name="op")
                        for dk in range(PAIRS):
                            nc.tensor.matmul(
                                ps[:],
                                aout[dk][:, msl],
                                wo_sb[:, dk, nsl],
                                start=(dk == 0),
                                stop=(dk == PAIRS - 1),
                            )
                        ost = ostp.tile([128, 512], F32, tag="ost")
                        nc.vector.tensor_add(ost[:], ps[:], bo_bc[:, nsl])
                        nc.sync.dma_start(out_d[msl, nsl], ost[:])

                    og = [(m, n) for m in range(MC // 128)
                          for n in range(2)]
                    qproj_x(0)
                    for nt in range(PAIRS):
                        qproj_group(0, nt)
                    leftover = []
                    for mc in range(NMC):
                        for pair in range(PAIRS):
                            if pair == 0 and mc == 0:
                                qproj_x(1)
                            work = []
                            if mc >= 1:
                                items = og[2 * pair:2 * pair + 2]
                                for k, (m_l, n_) in enumerate(items):
                                    mt = (mc - 1) * (MC // 128) + m_l
                                    fn = (lambda mtt=mt, nn=n_:
                                          outproj_item(mtt, nn))
                                    # reserve a couple of items to keep the
                                    # PE fed through the final divide
                                    if mc == NMC - 1 and pair >= 2 and k:
                                        leftover.append(fn)
                                    else:
                                        work.append(fn)
                            if mc == 0:
                                work.append(
                                    lambda nt=pair: qproj_group(1, nt))
                            attention(mc, pair, work,
                                      fast_divide=(mc == NMC - 1
                                                   and pair >= PAIRS - 2))
                    for fn in leftover:
                        fn()
                    for m_l in range(MC // 128):
                        for n_ in range(2):
                            outproj_item(12 + m_l, n_)

    n = _dedup_ldweights(nc)
    print(f"dedup_ldweights removed {n}")
    return nc


def kernel(**inputs):
    query = np.asarray(inputs["query"], np.float32)
    key = np.asarray(inputs["key"], np.float32)
    value = np.asarray(inputs["value"], np.float32)
    Wq = np.asarray(inputs["Wq"], np.float32)
    bq = np.asarray(inputs["bq"], np.float32)
    Wk = np.asarray(inputs["Wk"], np.float32)
    Wv = np.asarray(inputs["Wv"], np.float32)
    bv = np.asarray(inputs["bv"], np.float32)
    Wo = np.asarray(inputs["Wo"], np.float32)
    bo = np.asarray(inputs["bo"], np.float32)

    nc = build_nc()

    in_maps = []
    for c in range(8):
        b, hh = c // 2, c % 2
        hs = slice(hh * HS, (hh + 1) * HS)

        def prep(a):
            return np.ascontiguousarray(a).astype(np.float16)

        bo_eff = bo * 0.5 + Wo[:, hs] @ bv[hs]
        in_maps.append({
            "xq_t": prep(query[b].T),
            "xk_t": prep(key[b].T),
            "xv_t": prep(value[b].T),
            "wq_t": prep(Wq[hs, :].T),
            "wk_t": prep(Wk[hs, :].T),
            "wv_t": prep(Wv[hs, :].T),
            "wo_t": prep(Wo[:, hs].T),
            "bq": np.ascontiguousarray(bq[hs]),
            "bo_row": bo_eff.reshape(1, E).astype(np.float32),
        })

    from concourse.bass_utils import run_bass_kernel_spmd
    nc.finalize()
    r = run_bass_kernel_spmd(nc, in_maps, core_ids=list(range(8)))
    globals()["LAST_RUN"] = r
    outs = [r.results[c]["out_partial"] for c in range(8)]
    return np.stack([outs[2 * b] + outs[2 * b + 1] for b in range(B)])
